# revision 48
# baseline (speedup 1.0000x reference)
"""Two-layer GCN (BongardGNN) on 8 Trainium2 NeuronCores.

No usable data-dependent-addressing primitive exists in this toolchain,
so the kernel runs as three dense device launches with host-side,
index-only reshuffles between them (the host never does arithmetic on
tensor values — it only sorts/pads/gathers/duplicates by the static
edge_index; the degree-derived normalizers dis = 1/sqrt(1+deg) are pure
index data and are computed on host):

  P1 (8 cores): q0 = dis*x -> bf16
  host: gather q0 rows into degree-bucketed slot-pair planes
        (self-loop included as slot 0, so no separate self term)
  P2 (8 cores): PE-centred pipeline — the slot reduce and the W1 matmul
                are ONE accumulating matmul chain per 448-column chunk:
                partition p = (g<4, d<2, f<16), lhsT = block-diag W1
                replicated over (g, d); ceil(cap/2) passes accumulate
                into PSUM f32.  relu on scalar; W2 block matmul (lagged
                2 chunks so the in-order PE queue never stalls on the
                scalar relu); q2 = dis^2 * (.) on DVE (relu positive
                homogeneity folds both GCN norm scalings past the relu
                when b1 == 0).  A DVE halving-tree path for a
                low-degree column tail exists (K2T > 0) but is off —
                measured, it traded PE time for DVE serial chains at no
                net win.
  P3 (8 cores): same PE-reduce with a delta-pattern lhsT over
                p = (g<16, d<4, f<2): out = dis*(sum_d mg2) (+ b2).

Performance structure (per core, memory-regime): the mg1 stream
(~14 MB bf16, ~39 us at the ~367 GB/s effective HBM rate) is the
roofline; PE consumes it at ~1 col/cycle fully overlapped; DVE/scalar
do only O(N) column scalings.  Nodes are globally degree-sorted so
per-column slot capacity is tight (~3% pad).  DMA bands are issued on
a single HWDGE ring (FIFO arrivals match consumption order; each ring
generates ~1 descriptor/20 ns, so 128-partition transfers want >=7 KB
lines), with small lead-in bands so the first matmul starts early.
Each launch pays ~8 us of counted fixed tail (NEFF semaphore-zeroing
postamble) on top of ~6 us of uncounted preamble.
"""

import os
import sys
import types

import numpy as np
import ml_dtypes
import concourse.bacc as bacc
import concourse.tile as tile
from concourse import mybir
from concourse.bass_utils import run_bass_kernel_spmd

F32 = mybir.dt.float32
BF16 = mybir.dt.bfloat16
NPBF = ml_dtypes.bfloat16

TRACE = bool(os.environ.get("GNN_TRACE"))
LAST_EXEC_NS = []


def _enable_tracing():
    """Register the axon NTFF profile hook (absent from this image's antenv)
    and stub out the slow artifact upload. Test-time only (GNN_TRACE=1)."""
    if "antenv.axon_hooks" not in sys.modules:
        mod = types.ModuleType("antenv.axon_hooks")
        state = {}
        mod.set_axon_ntff_profile_hook = lambda h: state.update(h=h)
        mod.get_axon_ntff_profile_hook = lambda: state.get("h")
        sys.modules["antenv.axon_hooks"] = mod
        import antenv

        antenv.axon_hooks = mod
        sys.path.insert(0, "/root/.axon_site")
        from trn_agent_boot.trn_boot import _ntff_profile_via_ctypes

        mod.set_axon_ntff_profile_hook(
            _ntff_profile_via_ctypes("/opt/axon/libaxon_pjrt.so")
        )
    import concourse.bass_utils as bu

    bu.upload_artifacts = lambda tmpdir: "skipped"


def _run(nc, in_maps, core_ids):
    if TRACE:
        _enable_tracing()
        res = run_bass_kernel_spmd(nc, in_maps, core_ids=core_ids, trace=True)
        LAST_EXEC_NS.append(res.exec_time_ns)
        return res
    return run_bass_kernel_spmd(nc, in_maps, core_ids=core_ids)


N = 200000
NCORES = 8
D0, D1, D2 = 16, 32, 2
CORE_IDS = list(range(NCORES))

NPAD = 200704  # 128 * 1568

# P1 grid: node = p*P1K + k per core, 25088 nodes per core
P1K = 196
NPC = NPAD // NCORES   # 25088

# P2 grid, PE path: 4 groups x K2PE node columns; rank = k*32 + g*8 + c.
# Tree path (low-degree tail): 8 groups x K2T columns, feature-major;
# rank = 32*K2PE + kt*64 + g*8 + c.  q2/dq column space is
# [0,K2PE) PE | [K2PE, K2PE+K2T) tree h=0 | [+K2T, +2*K2T) tree h=1.
K2 = NPC // 4          # 6272 (total q2 columns)
K2PE = 6272
K2T = (K2 - K2PE) // 2  # 1344
CH2 = 512              # columns per matmul chunk (full PSUM bank)

# P3 grid: 16 groups x K3 node columns per core; rank = k*128 + g*8 + c
K3 = NPC // 16         # 1568
CH3 = 392              # 4 chunks


def _ceil(a, m):
    return -((-a) // m) * m


def _runs(caps):
    """Maximal (k0, k1, cap) runs of equal capacity."""
    runs = []
    k0 = 0
    for k in range(1, len(caps) + 1):
        if k == len(caps) or caps[k] != caps[k0]:
            runs.append((k0, k, int(caps[k0])))
            k0 = k
    return runs


def _merged_runs(caps, max_extra):
    """Runs of equal cap, greedily merging a run into its (higher-cap)
    predecessor when the extra padded slots cost less than the saved
    per-run overhead."""
    runs = _runs(caps)
    out = [list(runs[0])]
    for k0, k1, v in runs[1:]:
        p0, p1, pv = out[-1]
        if (k1 - k0) * (pv - v) <= max_extra:
            out[-1][1] = k1
        else:
            out.append([k0, k1, v])
    return [(a, b, v) for a, b, v in out]


def _plan_chunks(jobs, chunk, maxcols=3584):
    """jobs: list of (k0, k1, v, slot_div, is_tree).  Split each into
    <=chunk column pieces (limited so one piece spans <= maxcols mg
    columns); return list of (k0, L, v, coloff, is_tree) in order, plus
    the total mg column count.  A piece spans (v // slot_div) * L mg
    columns."""
    chunks = []
    off = 0
    for k0, k1, v, slot_div, is_tree in jobs:
        L = k1 - k0
        ceff = min(chunk, max(64, maxcols // (v // slot_div)))
        nch = -(-L // ceff)
        base = k0
        for i in range(nch):
            c0 = base + (L * i) // nch
            c1 = base + (L * (i + 1)) // nch
            Lc = c1 - c0
            chunks.append((c0, Lc, v, off, is_tree))
            off += (v // slot_div) * Lc
    return chunks, off


def build_p1():
    """q0 = disb * x -> bf16, 25088 nodes per core."""
    nc = bacc.Bacc("TRN2", target_bir_lowering=False, debug=False)
    xc = nc.dram_tensor("xc", [128, P1K * D0], F32, kind="ExternalInput")
    disc = nc.dram_tensor("disc", [128, P1K], BF16, kind="ExternalInput")
    q0o = nc.dram_tensor("q0o", [128, P1K * D0], BF16, kind="ExternalOutput")

    with tile.TileContext(nc) as tc:
        with tc.tile_pool(name="pool", bufs=4) as pool, tc.tile_pool(
            name="cpool", bufs=1
        ) as cpool:
            disb = cpool.tile([128, P1K], BF16)
            nc.gpsimd.dma_start(out=disb[:], in_=disc[:])
            NCH = 4
            KC = P1K // NCH
            xts = []
            for u in range(NCH):
                # alternate the two HWDGE rings: each ring's descriptor
                # generator is the per-DMA rate limit at small line sizes
                xt = pool.tile([128, KC * D0], F32, tag="xt")
                (nc.sync, nc.scalar)[u % 2].dma_start(
                    out=xt[:], in_=xc[:, u * KC * D0:(u + 1) * KC * D0]
                )
                xts.append(xt)
            for u in range(NCH):
                xt = xts[u]
                q0t = pool.tile([128, KC * D0], BF16, tag="q0t")
                nc.vector.tensor_tensor(
                    out=q0t[:].rearrange("p (k f) -> p k f", f=D0),
                    in0=xt[:].rearrange("p (k f) -> p k f", f=D0),
                    in1=disb[:, u * KC:(u + 1) * KC]
                    .rearrange("p (k o) -> p k o", o=1)
                    .to_broadcast([128, KC, D0]),
                    op=mybir.AluOpType.mult,
                )
                (nc.scalar, nc.sync)[u % 2].dma_start(
                    out=q0o[:, u * KC * D0:(u + 1) * KC * D0], in_=q0t[:]
                )
    nc.compile()
    return nc


def build_p2(chunks2, smg, homog):
    """Hybrid slot-reduce + fused GCN matmuls, per 448-col chunk.

    PE-path chunk (c0, L, v, off, False): passes j = 0..v/2-1, each a
    [128, L] plane at columns off + j*L; partition p = g*32 + d*16 + f
    carries q0[slot[node(g, k), 2j+d], f]; the accumulating matmul
    chain IS the reduce and the W1 matmul.

    Tree-path chunk (c0, L, v, off, True): v d-major planes [128, L],
    partition p = g*16 + f (8 groups, feature-major); an in-place DVE
    bf16 halving tree reduces them, then two 64-partition block-diag W1
    matmuls (h-halves) produce the same [128 = 4 groups x 32h, L]
    layout as the PE path.  Low-degree columns go here to keep the PE
    under the DMA stream rate.

    homog=True (b1 == 0): relu(W1^T agg) plain, both dis scalings fold
    into one dis^2 multiply on the [8, L] W2 output (relu positive
    homogeneity).  homog=False: scale PSUM by dis before relu+bias and
    the W2 output by dis.
    """
    nc = bacc.Bacc("TRN2", target_bir_lowering=False, debug=False)
    mg1 = nc.dram_tensor("mg1", [128, smg], BF16, kind="ExternalInput")
    w1d = nc.dram_tensor("w1d", [128, 128], BF16, kind="ExternalInput")
    w1td = nc.dram_tensor("w1td", [128, 128], BF16, kind="ExternalInput")
    w2d = nc.dram_tensor("w2d", [128, 8], BF16, kind="ExternalInput")
    dqd = nc.dram_tensor("dqd", [8, K2], BF16, kind="ExternalInput")
    q2d = nc.dram_tensor("q2d", [8, K2], BF16, kind="ExternalOutput")
    if not homog:
        dfd = nc.dram_tensor("dfd", [128, K2], BF16, kind="ExternalInput")
        b1d = nc.dram_tensor("b1d", [128, 1], F32, kind="ExternalInput")

    # group chunks into DMA bands; first bands small so compute starts early
    def _bands_of(chunks, sizes):
        bands = []
        bi = 0
        for ch in chunks:
            sdiv = 1 if ch[4] else 2
            ncols = (ch[2] // sdiv) * ch[1]
            lim = sizes[min(bi, len(sizes) - 1)]
            if bands and bands[-1][1] + ncols <= lim:
                bands[-1][1] += ncols
                bands[-1][2].append(ch)
            else:
                bands.append([ch[3], ncols, [ch]])
                bi += 1
        return bands

    bands = _bands_of(chunks2, [1024, 2048, 4096, 7168])
    BANDMAX = max(b[1] for b in bands)

    with tile.TileContext(nc) as tc:
        with (
            tc.tile_pool(name="pool", bufs=6) as pool,
            tc.tile_pool(name="hpool", bufs=6) as hpool,
            tc.tile_pool(name="cpool", bufs=1) as cpool,
            tc.tile_pool(name="psum", bufs=3, space="PSUM") as psum,
            tc.tile_pool(name="ps2p", bufs=3, space="PSUM") as ps2p,
        ):
            w1b = cpool.tile([128, 128], BF16)
            w1t = cpool.tile([128, 128], BF16)
            w2b = cpool.tile([128, 8], BF16)
            dqs = cpool.tile([8, K2], BF16)
            q2s = cpool.tile([8, K2], BF16)
            if not homog:
                dfs = cpool.tile([128, K2], BF16)
                b1s = cpool.tile([128, 1], F32)

            nc.gpsimd.dma_start(out=w1b[:], in_=w1d[:])
            nc.gpsimd.dma_start(out=w1t[:], in_=w1td[:])
            nc.gpsimd.dma_start(out=w2b[:], in_=w2d[:])
            nc.gpsimd.dma_start(out=dqs[:], in_=dqd[:])
            if not homog:
                nc.gpsimd.dma_start(out=dfs[:], in_=dfd[:])
                nc.gpsimd.dma_start(out=b1s[:], in_=b1d[:])

            qdone = [0]

            def emit_w2(c0, L, h1s):
                ps2 = ps2p.tile([8, CH2], F32, tag="ps2")
                nc.tensor.matmul(
                    out=ps2[:, :L],
                    lhsT=w2b[:],
                    rhs=h1s[:, :L],
                    start=True,
                    stop=True,
                )
                nc.vector.tensor_tensor(
                    out=q2s[:, c0:c0 + L],
                    in0=ps2[:, :L],
                    in1=dqs[:, c0:c0 + L],
                    op=mybir.AluOpType.mult,
                )
                if c0 + L <= K2PE and qdone[0] < K2 // 2 <= c0 + L:
                    nc.sync.dma_start(
                        out=q2d[:, :c0 + L], in_=q2s[:, :c0 + L]
                    )
                    qdone[0] = c0 + L

            def finish_chunk(qc0, L, ps1, dcol):
                """relu (+ optional dis scale/bias) -> queue for W2."""
                h1s = hpool.tile([128, CH2], BF16, tag="h1s")
                if homog:
                    nc.scalar.activation(
                        h1s[:, :L],
                        ps1[:, :L],
                        mybir.ActivationFunctionType.Relu,
                    )
                else:
                    s1 = hpool.tile([128, CH2], F32, tag="s1")
                    nc.vector.tensor_tensor(
                        out=s1[:, :L],
                        in0=ps1[:, :L],
                        in1=dfs[:, dcol:dcol + L],
                        op=mybir.AluOpType.mult,
                    )
                    nc.scalar.activation(
                        h1s[:, :L],
                        s1[:, :L],
                        mybir.ActivationFunctionType.Relu,
                        bias=b1s[:],
                    )
                pend.append((qc0, L, h1s))
                # lag-2 second matmul: its relu is long done -> no PE stall
                if len(pend) > 2:
                    emit_w2(*pend.pop(0))

            tree_pend = []

            def emit_tree_mm(c0, L, mgt_, loc_):
                for h in (0, 1):
                    ps1 = psum.tile([128, CH2], F32, tag="ps1")
                    nc.tensor.matmul(
                        out=ps1[:, :L],
                        lhsT=w1t[64 * h:64 * h + 64, :],
                        rhs=mgt_[64 * h:64 * h + 64, loc_:loc_ + L],
                        start=True,
                        stop=True,
                    )
                    finish_chunk(
                        K2PE + h * K2T + c0, L, ps1, K2PE + h * K2T + c0
                    )

            pend = []
            qi = 0
            for boff, bcols, bchunks in bands:
                mgt = pool.tile([128, BANDMAX], BF16, tag="mgt")
                # lead-in bands go on three parallel DMA paths so their
                # per-DMA descriptor floors overlap; the steady-state
                # stream stays on the sync ring (FIFO arrival order)
                eng = (nc.sync, nc.scalar, nc.gpsimd)[qi] if qi < 3 else nc.sync
                qi += 1
                eng.dma_start(out=mgt[:, :bcols], in_=mg1[:, boff:boff + bcols])
                for c0, L, v, off, is_tree in bchunks:
                    loc = off - boff
                    if not is_tree:
                        ps1 = psum.tile([128, CH2], F32, tag="ps1")
                        vh = v // 2
                        for j in range(vh):
                            nc.tensor.matmul(
                                out=ps1[:, :L],
                                lhsT=w1b[:],
                                rhs=mgt[:, loc + j * L:loc + (j + 1) * L],
                                start=(j == 0),
                                stop=(j == vh - 1),
                            )
                        finish_chunk(c0, L, ps1, c0)
                        # tree W1 matmuls lag one PE chunk so the PE queue
                        # never waits on the DVE halving tree
                        while tree_pend:
                            emit_tree_mm(*tree_pend.pop(0))
                        continue
                    # tree path: in-place bf16 halving tree over v planes
                    cur = v
                    while cur > 1:
                        if cur % 2:
                            nc.vector.tensor_tensor(
                                out=mgt[:, loc:loc + L],
                                in0=mgt[:, loc:loc + L],
                                in1=mgt[:, loc + (cur - 1) * L:loc + cur * L],
                                op=mybir.AluOpType.add,
                            )
                            cur -= 1
                        h2 = cur // 2
                        nc.vector.tensor_tensor(
                            out=mgt[:, loc:loc + h2 * L],
                            in0=mgt[:, loc:loc + h2 * L],
                            in1=mgt[:, loc + h2 * L:loc + 2 * h2 * L],
                            op=mybir.AluOpType.add,
                        )
                        cur = h2
                    tree_pend.append((c0, L, mgt, loc))
            while tree_pend:
                emit_tree_mm(*tree_pend.pop(0))
            for args in pend:
                emit_w2(*args)
            nc.sync.dma_start(
                out=q2d[:, qdone[0]:], in_=q2s[:, qdone[0]:]
            )
    nc.compile()
    return nc


def build_p3(chunks3, smg, with_bias):
    """out = dis*(sum_d mg2) (+ b2): PE-reduce, p = (g<16, d<4, f<2)."""
    nc = bacc.Bacc("TRN2", target_bir_lowering=False, debug=False)
    mg2 = nc.dram_tensor("mg2", [128, smg], BF16, kind="ExternalInput")
    eyd = nc.dram_tensor("eyd", [128, 32], BF16, kind="ExternalInput")
    dld = nc.dram_tensor("dld", [32, K3], BF16, kind="ExternalInput")
    out3 = nc.dram_tensor("out3", [32, K3], F32, kind="ExternalOutput")
    if with_bias:
        b2d = nc.dram_tensor("b2d", [32, 1], F32, kind="ExternalInput")

    bands = []
    bi = 0
    sizes = [1024, 2048, 4608]
    for ch in chunks3:
        ncols = (ch[2] // 4) * ch[1]
        lim = sizes[min(bi, len(sizes) - 1)]
        if bands and bands[-1][1] + ncols <= lim:
            bands[-1][1] += ncols
            bands[-1][2].append(ch)
        else:
            bands.append([ch[3], ncols, [ch]])
            bi += 1
    BANDMAX = max(b[1] for b in bands)

    with tile.TileContext(nc) as tc:
        with (
            tc.tile_pool(name="pool", bufs=3) as pool,
            tc.tile_pool(name="cpool", bufs=1) as cpool,
            tc.tile_pool(name="psum", bufs=3, space="PSUM") as psum,
        ):
            eye = cpool.tile([128, 32], BF16)
            dls = cpool.tile([32, K3], BF16)
            outs = cpool.tile([32, K3], F32)
            if with_bias:
                b2s = cpool.tile([32, 1], F32)

            nc.gpsimd.dma_start(out=eye[:], in_=eyd[:])
            nc.gpsimd.dma_start(out=dls[:], in_=dld[:])
            if with_bias:
                nc.gpsimd.dma_start(out=b2s[:], in_=b2d[:])
            done = 0
            qi = 0
            for boff, bcols, bchunks in bands:
                mgt = pool.tile([128, BANDMAX], BF16, tag="mgt")
                eng = (nc.sync, nc.scalar, nc.gpsimd)[qi % 3]
                qi += 1
                eng.dma_start(
                    out=mgt[:, :bcols], in_=mg2[:, boff:boff + bcols]
                )
                for c0, L, v, off, _ in bchunks:
                    loc = off - boff
                    ps = psum.tile([32, CH3], F32, tag="ps")
                    vq = v // 4
                    for j in range(vq):
                        nc.tensor.matmul(
                            out=ps[:, :L],
                            lhsT=eye[:],
                            rhs=mgt[:, loc + j * L:loc + (j + 1) * L],
                            start=(j == 0),
                            stop=(j == vq - 1),
                        )
                    if with_bias:
                        sc = pool.tile([32, CH3], F32, tag="sc")
                        nc.vector.tensor_tensor(
                            out=sc[:, :L],
                            in0=ps[:, :L],
                            in1=dls[:, c0:c0 + L],
                            op=mybir.AluOpType.mult,
                        )
                        nc.scalar.activation(
                            outs[:, c0:c0 + L],
                            sc[:, :L],
                            mybir.ActivationFunctionType.Copy,
                            bias=b2s[:],
                        )
                    else:
                        nc.vector.tensor_tensor(
                            out=outs[:, c0:c0 + L],
                            in0=ps[:, :L],
                            in1=dls[:, c0:c0 + L],
                            op=mybir.AluOpType.mult,
                        )
                    done += 1
                    if done in (len(chunks3) // 2, len(chunks3)):
                        o0 = 0 if done == len(chunks3) // 2 else done_half
                        nc.sync.dma_start(
                            out=out3[:, o0:c0 + L], in_=outs[:, o0:c0 + L]
                        )
                        done_half = c0 + L
    nc.compile()
    return nc


def kernel(x, edge_index, W1, b1, W2, b2):
    LAST_EXEC_NS.clear()
    x = np.asarray(x, np.float32)
    W1 = np.asarray(W1, np.float32)
    b1 = np.asarray(b1, np.float32)
    W2 = np.asarray(W2, np.float32)
    b2 = np.asarray(b2, np.float32)
    src = np.asarray(edge_index[0], np.int64)
    dst = np.asarray(edge_index[1], np.int64)
    homog = not np.any(b1)
    with_bias = bool(np.any(b2))

    # ---- host index prep (degree data only) ----
    deg = np.bincount(dst, minlength=N).astype(np.int64)
    dis = 1.0 / np.sqrt(1.0 + deg.astype(np.float32))
    disb = dis.astype(NPBF)
    capmax = _ceil(int(deg.max()) + 1, 4)
    order_e = np.argsort(dst, kind="stable")
    s_src = src[order_e]
    s_dst = dst[order_e]
    starts = np.zeros(N + 1, np.int64)
    np.cumsum(deg, out=starts[1:])
    slot = np.full((N + 1, capmax), N, np.int64)  # row N = zero sentinel
    slot[:N, 0] = np.arange(N)                   # self-loop slot
    pos = np.arange(len(s_src)) - starts[s_dst]
    slot[s_dst, pos + 1] = s_src

    onode = np.argsort(-deg, kind="stable")
    order_ext = np.concatenate([onode, np.full(NPAD - N, N, np.int64)])
    deg_ext = np.concatenate([deg[onode] + 1, np.ones(NPAD - N, np.int64)])
    dis_ext = np.concatenate([dis[onode], np.ones(NPAD - N, np.float32)])

    NPE = 32 * K2PE
    nodes2 = order_ext[:NPE].reshape(K2PE, 4, NCORES)   # [k, g, c] PE path
    nodest = order_ext[NPE:].reshape(K2T, 8, NCORES)    # [kt, g, c] tree path
    caps2 = np.maximum(_ceil(deg_ext[:NPE:32], 2), 2)
    capst = np.maximum(_ceil(deg_ext[NPE::64], 2), 2)
    runs2 = _merged_runs(caps2, max_extra=1024)
    runst = _merged_runs(capst, max_extra=1024) if K2T else []
    # weave tree runs proportionally among PE runs so the DVE tree work
    # spreads over the whole stream; keep a PE run last (short tail)
    pe_jobs = [(k0, k1, v, 2, False) for k0, k1, v in runs2]
    tr_jobs = [(k0, k1, v, 1, True) for k0, k1, v in runst]
    pe_cols = [(v // 2) * (k1 - k0) for k0, k1, v in runs2]
    tr_cols = [v * (k1 - k0) for k0, k1, v in runst]
    tot_pe, tot_tr = sum(pe_cols), sum(tr_cols)
    jobs2 = []
    acc_pe = acc_tr = 0
    ti = 0
    for j, job in enumerate(pe_jobs[:-1]):
        jobs2.append(job)
        acc_pe += pe_cols[j]
        while ti < len(tr_jobs) and acc_tr * tot_pe < acc_pe * tot_tr:
            jobs2.append(tr_jobs[ti])
            acc_tr += tr_cols[ti]
            ti += 1
    jobs2 += tr_jobs[ti:]
    jobs2.append(pe_jobs[-1])
    chunks2, smg2 = _plan_chunks(jobs2, CH2, maxcols=4608)

    nodes3 = order_ext.reshape(K3, 16, NCORES)   # [k, g, c]
    caps3 = np.maximum(_ceil(deg_ext[::128], 4), 4)
    runs3 = _merged_runs(caps3, max_extra=1024)
    jobs3 = [(k0, k1, v, 4, False) for k0, k1, v in runs3]
    chunks3, smg3 = _plan_chunks(jobs3, CH3)

    # ---- P1: q0(bf16) on 8 cores ----
    p1 = build_p1()
    in1 = []
    for c in range(NCORES):
        sl = slice(c * (N // NCORES), (c + 1) * (N // NCORES))
        xp = np.zeros((NPC, D0), np.float32)
        xp[:N // NCORES] = x[sl]
        dd = np.ones(NPC, np.float32).astype(NPBF)
        dd[:N // NCORES] = disb[sl]
        in1.append(
            {
                "xc": np.ascontiguousarray(xp.reshape(128, P1K * D0)),
                "disc": np.ascontiguousarray(dd.reshape(128, P1K)),
            }
        )
    r1 = _run(p1, in1, core_ids=CORE_IDS).results
    q0ext = np.zeros((N + 1, D0), NPBF)
    for c in range(NCORES):
        sl = slice(c * (N // NCORES), (c + 1) * (N // NCORES))
        q0ext[sl] = np.asarray(r1[c]["q0o"]).reshape(NPC, D0)[:N // NCORES]

    # ---- host join 1: build mg1 (slot-pair planes + tree planes) ----
    p2 = build_p2(chunks2, smg2, homog)
    w1blk = np.zeros((128, 128), np.float32)
    w1tbk = np.zeros((128, 128), np.float32)
    w2blk = np.zeros((128, 8), np.float32)
    for g in range(4):
        for d in range(2):
            w1blk[g * 32 + d * 16:g * 32 + d * 16 + 16, g * 32:g * 32 + 32] = W1
        w2blk[g * 32:g * 32 + 32, g * 2:g * 2 + 2] = W2
    for g in range(8):
        w1tbk[g * 16:g * 16 + 16, (g % 4) * 32:(g % 4) * 32 + 32] = W1
    w1blk = w1blk.astype(NPBF)
    w1tbk = w1tbk.astype(NPBF)
    w2blk = w2blk.astype(NPBF)
    pw = 2.0 if homog else 1.0
    in2 = []
    for c in range(NCORES):
        grid = nodes2[:, :, c].T          # [4, K2PE] node ids
        gridt = nodest[:, :, c].T         # [8, K2T]
        dgrid = dis_ext[:NPE].reshape(K2PE, 4, NCORES)[:, :, c].T
        dgrdt = dis_ext[NPE:].reshape(K2T, 8, NCORES)[:, :, c].T  # [8, K2T]
        mg1 = np.empty((128, smg2), NPBF)
        for c0, L, v, off, is_tree in chunks2:
            if not is_tree:
                idx = slot[grid[:, c0:c0 + L], :v]       # [4, L, v]
                gq = q0ext[idx]                          # [4, L, v, 16]
                # -> [g, d(2), f(16), j(v/2), k(L)]
                arr = gq.reshape(4, L, v // 2, 2, 16).transpose(0, 3, 4, 2, 1)
                mg1[:, off:off + (v // 2) * L] = arr.reshape(128, -1)
            else:
                idx = slot[gridt[:, c0:c0 + L], :v]      # [8, L, v]
                gq = q0ext[idx]                          # [8, L, v, 16]
                arr = gq.transpose(0, 3, 2, 1)           # [g, f, d, k]
                mg1[:, off:off + v * L] = arr.reshape(128, -1)
        # dq columns: [PE | tree h=0 | tree h=1]
        dq_pe = np.repeat((dgrid ** pw).astype(NPBF)[:, None, :], 2, axis=1
                          ).reshape(8, K2PE)
        dt = (dgrdt ** pw).astype(NPBF)                  # [8, K2T]
        dq_t = [
            np.repeat(dt[4 * h:4 * h + 4][:, None, :], 2, axis=1).reshape(
                8, K2T
            )
            for h in (0, 1)
        ]
        d = {
            "mg1": mg1,
            "w1d": w1blk,
            "w1td": w1tbk,
            "w2d": w2blk,
            "dqd": np.ascontiguousarray(
                np.concatenate([dq_pe] + dq_t, axis=1)
            ),
        }
        if not homog:
            df_pe = np.repeat(
                dgrid.astype(NPBF)[:, None, :], 32, axis=1
            ).reshape(128, K2PE)
            df_t = [
                np.repeat(
                    dgrdt[4 * h:4 * h + 4].astype(NPBF)[:, None, :], 32, axis=1
                ).reshape(128, K2T)
                for h in (0, 1)
            ]
            d["dfd"] = np.ascontiguousarray(
                np.concatenate([df_pe] + df_t, axis=1)
            )
            d["b1d"] = np.ascontiguousarray(np.tile(b1, 4).reshape(128, 1))
        in2.append(d)
    r2 = _run(p2, in2, core_ids=CORE_IDS).results
    q2ext = np.zeros((N + 1, D2), NPBF)
    for c in range(NCORES):
        q2c = np.asarray(r2[c]["q2d"])
        vals = q2c[:, :K2PE].reshape(4, 2, K2PE)           # [g, j, k]
        q2ext[nodes2[:, :, c].T] = vals.transpose(0, 2, 1)  # [4, K2PE, 2]
        for h in (0, 1):
            blk = q2c[:, K2 - (2 - h) * K2T:K2 - (1 - h) * K2T]
            blk = blk.reshape(4, 2, K2T)                   # [g', j, kt]
            q2ext[nodest[:, 4 * h:4 * h + 4, c].T] = blk.transpose(0, 2, 1)
    q2ext[N] = 0

    # ---- host join 2: build mg2 (slot-quad planes) ----
    p3 = build_p3(chunks3, smg3, with_bias)
    eyd = np.zeros((128, 32), np.float32)
    for g in range(16):
        for d in range(4):
            for f in range(2):
                eyd[g * 8 + d * 2 + f, g * 2 + f] = 1.0
    eyd = eyd.astype(NPBF)
    in3 = []
    for c in range(NCORES):
        grid = nodes3[:, :, c].T          # [16, K3]
        dgrid = dis_ext.reshape(K3, 16, NCORES)[:, :, c].T
        mg2 = np.empty((128, smg3), NPBF)
        for c0, L, v, off, _ in chunks3:
            idx = slot[grid[:, c0:c0 + L], :v]       # [16, L, v]
            gq = q2ext[idx]                          # [16, L, v, 2]
            arr = gq.reshape(16, L, v // 4, 4, 2).transpose(0, 3, 4, 2, 1)
            mg2[:, off:off + (v // 4) * L] = arr.reshape(128, -1)
        dl = np.repeat(dgrid.astype(NPBF)[:, None, :], 2, axis=1)
        d = {
            "mg2": mg2,
            "eyd": eyd,
            "dld": np.ascontiguousarray(dl.reshape(32, K3)),
        }
        if with_bias:
            d["b2d"] = np.ascontiguousarray(np.tile(b2, 16).reshape(32, 1))
        in3.append(d)
    r3 = _run(p3, in3, core_ids=CORE_IDS).results
    outfull = np.zeros((N + 1, D2), np.float32)
    for c in range(NCORES):
        vals = np.asarray(r3[c]["out3"]).reshape(16, 2, K3)  # [g, f, k]
        outfull[nodes3[:, :, c].T] = vals.transpose(0, 2, 1)
    return np.ascontiguousarray(outfull[:N])
